# revision 36
# baseline (speedup 1.0000x reference)
"""2-layer GAT (PyG GATConv x2) on 8 Trainium2 NeuronCores via Bass/Tile.

Strategy (self-contained; shapes hardcoded for the nn_GAT problem):
  - nodes split 2500/core (dst-sharded aggregation); edges (+self-loops)
    sorted by dst; per-core edge stream padded to an SPMD-uniform schedule
    of 128-edge windows grouped in 20 dst-tiles of 125 dst nodes.
  - layer 1 is FUSED: the host permutes x into edge-slot order and
    pre-transposes it (xT_perm [128, EPAD]); each window computes its
    messages h_e = x_e @ W1 directly on the PE (no h-table, no gather).
    a_s rides the same operand via a second matmul against W_att; a_d is
    expanded dst->edges on the PE via host-built S^T one-hot matmuls.
    No segment-max is needed: logits are small and softmax is
    shift-invariant (validated vs fp64 ground truth by the baseline).
  - scatter-aggregate: per 128-edge window, scale messages by exp (DVE
    broadcast-mul straight out of PSUM), one-hot matmul (host-built S,
    zeroed at pad slots) accumulating numerator [125,512] and denominator
    [125,8] in PSUM; then divide, bias, relu.
  - layer 2: h2 = relu(out1)@W2 per dst-shard, packed with a_s2 into a
    [2500,128] bf16 table piece, AllGather'd; per-window dma_gather by
    src (the only indexed-DMA pass; GpSimd descriptor-gen is the scarce
    resource) with 64 ch / 1 head; output written dst-sharded and
    concatenated on host.
"""

import os
import sys

sys.path.insert(0, os.path.dirname(os.path.abspath(__file__)))
try:
    import axon_shim
    axon_shim.install()
except Exception:
    pass

import numpy as np
import ml_dtypes

import concourse.bacc as bacc
import concourse.bass as bass
import concourse.mybir as mybir
import concourse.tile as tile
from concourse import library_config
from concourse.tile import add_dep_helper
from concourse.bass_utils import run_bass_kernel_spmd

F32 = mybir.dt.float32
BF16 = mybir.dt.bfloat16
I16 = mybir.dt.int16

N, E, IN, HID, HEADS, OUT = 20000, 320000, 128, 64, 8, 64
NEG = 0.2
NCORES = 8
NPC = N // NCORES          # 2500 nodes per core
TILE_D = 125               # dst nodes per tile
NT = NPC // TILE_D         # 20 tiles per core
NROWS = N + 4              # pad row N holds "neutral" values
PAD = N                    # pad row index
CH1 = HEADS * HID          # 512
WCH = 8                    # windows per gather chunk (1024 idx)
BIG = -1.0e4               # pad-row a_s value -> exp(lrelu(...)) == 0


# ----------------------------------------------------------------- host prep
NQ = 4                     # src quarters / partial-AllGather waves
# group q = tiles [QT[q], QT[q+1]); small first group starts gathers early
QT = [0, 4, 9, 15, 20]
QSZ = [(QT[q + 1] - QT[q]) * TILE_D for q in range(NQ)]   # nodes/core/quarter
QSTART = [QT[q] * TILE_D for q in range(NQ)]
# stacked gathered-table row offset of quarter q
TOFF = np.concatenate([[0], np.cumsum([NCORES * s for s in QSZ])])
Q_PAD = int(TOFF[NQ])      # pad row right after the stacked tables


def preprocess(edge_index, x_bf16):
    src0 = edge_index[0].astype(np.int64)
    dst0 = edge_index[1].astype(np.int64)
    loop = np.arange(N, dtype=np.int64)
    src = np.concatenate([src0, loop])
    dst = np.concatenate([dst0, loop])
    order = np.argsort(dst, kind="stable")
    src, dst = src[order], dst[order]

    gtile = dst // TILE_D                       # global tile id, 0..159
    counts = np.bincount(gtile, minlength=NCORES * NT)
    W = np.zeros(NT, np.int64)
    for t in range(NT):
        W[t] = (counts[t::NT].max() + 127) // 128
    WOFF = np.zeros(NT + 1, np.int64)
    WOFF[1:] = np.cumsum(W)
    TW = int(WOFF[-1])
    EPAD = TW * 128

    nchunk = (TW + WCH - 1) // WCH
    chunk_w = [min(WCH, TW - c * WCH) for c in range(nchunk)]

    def idx_layout(a, cw_list):
        """pack int16 indices in per-chunk column-major-wrapped layout"""
        outb = []
        off = 0
        for cwn in cw_list:
            n_i = cwn * 128
            blk = a[off:off + n_i].astype(np.int16)
            outb.append(np.tile(blk.reshape(-1, 16).T.copy(), (8, 1)))
            off += n_i
        return np.concatenate(outb, axis=1)

    def build_sst(s_slots, dl_slots, real_slots):
        S = np.zeros((128, TW * 128), ml_dtypes.bfloat16)
        pos = np.arange(EPAD)
        S[pos[real_slots] % 128,
          (pos[real_slots] // 128) * 128 + dl_slots[real_slots]] = 1.0
        ST = np.zeros((128, TW * 128), ml_dtypes.bfloat16)
        ST[dl_slots[real_slots],
           (pos[real_slots] // 128) * 128 + (pos[real_slots] % 128)] = 1.0
        return S, ST

    # x rows with a zero pad row appended (pad slots gather row N == 0)
    xpad = np.concatenate(
        [x_bf16, np.zeros((1, IN), ml_dtypes.bfloat16)], axis=0)

    edge_off = np.zeros(NCORES * NT + 1, np.int64)
    edge_off[1:] = np.cumsum(counts)
    cores = []
    # L2 pass-major stream order (same for all cores by construction of W):
    # per tile, windows get a pass = src-quarter of their last real edge;
    # stream = sorted by (pass, tile, window). Window->pass differs per core,
    # but the SPMD program needs ONE order — use the max pass across cores so
    # every core's data for that window is available by that pass.
    wpass_cores = np.zeros((NCORES, TW), np.int64)
    src2_cores = []
    for c in range(NCORES):
        s_arr = np.full(EPAD, PAD, np.int64)
        dl_arr = np.zeros(EPAD, np.int64)
        real = np.zeros(EPAD, np.bool_)
        s2_arr = np.full(EPAD, -1, np.int64)     # L2 src per slot (quarter-sorted)
        dl2_arr = np.zeros(EPAD, np.int64)
        real2 = np.zeros(EPAD, np.bool_)
        for t in range(NT):
            g = c * NT + t
            cnt = counts[g]
            base = WOFF[t] * 128
            sl = slice(edge_off[g], edge_off[g + 1])
            s_arr[base:base + cnt] = src[sl]
            dl_arr[base:base + cnt] = dst[sl] - (c * NPC + t * TILE_D)
            real[base:base + cnt] = True
            # L2: re-sort this tile's edges by src quarter
            st_ = src[sl]
            qt = np.searchsorted(np.array(QSTART[1:] + [NPC]),
                                 st_ % NPC, side="right")
            o2 = np.argsort(qt, kind="stable")
            s2_arr[base:base + cnt] = st_[o2]
            dl2_arr[base:base + cnt] = (dst[sl] - (c * NPC + t * TILE_D))[o2]
            real2[base:base + cnt] = True
            qsorted = qt[o2]
            for w in range(int(W[t])):
                last = min(cnt - 1, (w + 1) * 128 - 1)
                wpass_cores[c, int(WOFF[t]) + w] = qsorted[last]
        S, ST = build_sst(s_arr, dl_arr, real)
        xTp = np.ascontiguousarray(xpad[s_arr].T)
        xTown = np.ascontiguousarray(x_bf16[c * NPC:(c + 1) * NPC].T)
        cores.append(dict(S=S, ST=ST, xTp=xTp, xTown=xTown))
        src2_cores.append((s2_arr, dl2_arr, real2))

    wpass = wpass_cores.max(axis=0)              # SPMD-uniform pass per window
    win_tile = np.zeros(TW, np.int64)
    for t in range(NT):
        win_tile[WOFF[t]:WOFF[t + 1]] = t
    # pass-major stream: list of original window ids
    order2 = [g for p in range(NQ) for g in range(TW) if wpass[g] == p]
    order2 = np.array(order2, np.int64)
    # per stream position: tile, segment start/stop, tile-final flag
    tile2 = win_tile[order2]
    st2 = np.zeros(TW, np.bool_)
    sp2 = np.zeros(TW, np.bool_)
    fin2 = np.zeros(TW, np.bool_)
    for j in range(TW):
        st2[j] = j == 0 or tile2[j - 1] != tile2[j]
        sp2[j] = j == TW - 1 or tile2[j + 1] != tile2[j]
        fin2[j] = order2[j] == WOFF[tile2[j] + 1] - 1   # tile's max window
    # chunk dep pass = pass of the chunk's last window (nondecreasing)
    chunk_pass = [int(wpass[order2[min(TW - 1, (ci + 1) * WCH - 1)]])
                  for ci in range(nchunk)]

    for c in range(NCORES):
        s2_arr, dl2_arr, real2 = src2_cores[c]
        # reorder per-window slot data into pass-major stream order
        s2s = np.concatenate([s2_arr[g * 128:(g + 1) * 128] for g in order2])
        dl2s = np.concatenate([dl2_arr[g * 128:(g + 1) * 128] for g in order2])
        real2s = np.concatenate([real2[g * 128:(g + 1) * 128] for g in order2])
        # map src node -> row in the stacked quarter tables
        rows = np.full(EPAD, Q_PAD, np.int64)
        rs = s2s[real2s]
        pos = rs % NPC
        q = np.searchsorted(np.array(QSTART[1:] + [NPC]), pos, side="right")
        rows[real2s] = (TOFF[q] + (rs // NPC) * np.array(QSZ)[q]
                        + (pos - np.array(QSTART)[q]))
        S2, ST2 = build_sst(s2s, dl2s, real2s)
        cores[c]["src_idx"] = idx_layout(rows, chunk_w)
        cores[c]["S2"] = S2
        cores[c]["ST2"] = ST2

    sched = dict(W=W, WOFF=WOFF, TW=TW, nchunk=nchunk, chunk_w=chunk_w,
                 tile2=tile2, st2=st2, sp2=sp2, fin2=fin2,
                 chunk_pass=chunk_pass)
    return sched, cores


# --------------------------------------------------------------- bass program
def build_program(sched):
    W, WOFF, TW = sched["W"], sched["WOFF"], sched["TW"]
    nchunk, chunk_w = sched["nchunk"], sched["chunk_w"]
    tile2, st2, sp2, fin2 = (sched["tile2"], sched["st2"], sched["sp2"],
                             sched["fin2"])
    chunk_pass = sched["chunk_pass"]
    win_tile = np.zeros(TW, np.int64)
    for t in range(NT):
        win_tile[WOFF[t]:WOFF[t + 1]] = t
    first_win = set(int(WOFF[t]) for t in range(NT))
    last_win = set(int(WOFF[t + 1] - 1) for t in range(NT))
    # L1 group boundary tiles: after closing tile GTILES[q], AllGather wave q
    GTILES = [QT[q + 1] - 1 for q in range(NQ)]

    nc = bacc.Bacc("TRN2", target_bir_lowering=False, debug=False,
                   num_devices=NCORES)

    # I/O (weights/constants pre-cast, pre-broadcast, pre-reduced on host)
    W1b_in = nc.dram_tensor("W1b", [IN, CH1], BF16, kind="ExternalInput")
    W2b_in = nc.dram_tensor("W2b", [128, 4 * OUT], BF16, kind="ExternalInput")
    wattb_in = nc.dram_tensor("wattb", [128, 16], BF16, kind="ExternalInput")
    b1bc_in = nc.dram_tensor("b1bc", [128, CH1], F32, kind="ExternalInput")
    b2bc_in = nc.dram_tensor("b2bc", [128, OUT], F32, kind="ExternalInput")
    att2sb_in = nc.dram_tensor("att2sb", [128, OUT], F32, kind="ExternalInput")
    att2db_in = nc.dram_tensor("att2db", [128, OUT], F32, kind="ExternalInput")
    ident_in = nc.dram_tensor("ident", [128, 128], BF16, kind="ExternalInput")
    srcidx_in = nc.dram_tensor("src_idx", [128, TW * 8], I16, kind="ExternalInput")
    S_in = nc.dram_tensor("S", [128, TW * 128], BF16, kind="ExternalInput")
    ST_in = nc.dram_tensor("ST", [128, TW * 128], BF16, kind="ExternalInput")
    S2_in = nc.dram_tensor("S2", [128, TW * 128], BF16, kind="ExternalInput")
    ST2_in = nc.dram_tensor("ST2", [128, TW * 128], BF16, kind="ExternalInput")
    xTp_in = nc.dram_tensor("xTp", [128, TW * 128], BF16, kind="ExternalInput")
    xTown_in = nc.dram_tensor("xTown", [128, NPC], BF16, kind="ExternalInput")
    y_out = nc.dram_tensor("y", [NPC, OUT], F32, kind="ExternalOutput")

    # internal DRAM
    t2piece = nc.dram_tensor("t2piece", [NPC, 128], BF16)
    # stacked quarter tables (+4 pad rows at the end)
    t2all = nc.dram_tensor("t2all", [Q_PAD + 4, 128], BF16,
                           addr_space="Shared")

    with tile.TileContext(nc, num_cores=NCORES) as tc:
        nc.gpsimd.load_library(library_config.mlp)
        with (
            tc.tile_pool(name="const", bufs=1) as constp,
            tc.tile_pool(name="work", bufs=2) as workp,
            tc.tile_pool(name="big", bufs=1) as bigp,
        ):
            # ---------------- phase 0: constants / setup ----------------
            w1b = constp.tile([128, CH1], BF16, tag="w1b")
            nc.sync.dma_start(w1b[:], W1b_in[:])
            w2b = constp.tile([128, 4, OUT], BF16, tag="w2b")
            nc.sync.dma_start(w2b[:].rearrange("p k n -> p (k n)"), W2b_in[:])
            identb = constp.tile([128, 128], BF16, tag="identb")
            nc.sync.dma_start(identb[:], ident_in[:])
            b1bc = constp.tile([128, CH1], F32, tag="b1bc")
            nc.sync.dma_start(b1bc[:], b1bc_in[:])
            b2bc = constp.tile([128, OUT], F32, tag="b2bc")
            nc.sync.dma_start(b2bc[:], b2bc_in[:])
            att2sb = constp.tile([128, OUT], F32, tag="att2sb")
            nc.sync.dma_start(att2sb[:], att2sb_in[:])
            att2db = constp.tile([128, OUT], F32, tag="att2db")
            nc.sync.dma_start(att2db[:], att2db_in[:])
            wattb = constp.tile([128, 16], BF16, tag="wattb")
            nc.sync.dma_start(wattb[:], wattb_in[:])

            # quarter-table pad rows: h2=0, a_s2=BIG (harmless; S2 zeroed
            # at pads) — every pad gather index points at Q_PAD
            prow = workp.tile([4, 128], BF16, tag="prow")
            nc.vector.memset(prow[:], 0.0)
            nc.vector.memset(prow[:, 64:72].bitcast(F32), BIG)
            nc.sync.dma_start(t2all[Q_PAD:Q_PAD + 4, :], prow[:])

            srcidx = bigp.tile([128, TW * 8], I16, tag="srcidx")
            nc.sync.dma_start(srcidx[:], srcidx_in[:])
            a2all = bigp.tile([TILE_D, NT, 2], F32, tag="a2all")
            # SBUF staging for ALL layer-2 gathered rows (lets gathers run
            # arbitrarily far ahead of PE consumption during layer 1)
            g2all = bigp.tile([128, TW, 128], BF16, tag="g2all")
            # SBUF accumulator for layer-2 numerator|denominator segments
            o2acc = bigp.tile([TILE_D, NT, OUT + 1], F32, tag="o2acc")
            nc.vector.memset(o2acc[:], 0.0)

            # ------------- phase 1: own-range a_d table -------------
            # adball[p, t, j] = a_d (j in 8:16) of own node t*125+p
            xTown = bigp.tile([128, NPC], BF16, tag="xTown")
            nc.sync.dma_start(xTown[:], xTown_in[:])
            adball = bigp.tile([TILE_D, NT, 16], BF16, tag="adball")
            with (
                tc.tile_pool(name="adp", bufs=2, space="PSUM") as adpp,
            ):
                for t in range(NT):
                    ps_ad = adpp.tile([TILE_D, 16], F32, tag="ps_ad")
                    nc.tensor.matmul(
                        ps_ad[:], xTown[:, t * TILE_D:(t + 1) * TILE_D],
                        wattb[:], start=True, stop=True)
                    nc.scalar.copy(adball[:, t, :], ps_ad[:])

            # ------------- phase 3: fused layer-1 + h2 -------------
            t2_writes = []
            gnops = []
            ccs = []

            def emit_wave(q):
                """AllGather wave q (GpSimd queue; deps via group nop)."""
                cc = nc.gpsimd.collective_compute(
                    "AllGather", mybir.AluOpType.bypass,
                    replica_groups=[list(range(NCORES))],
                    ins=[t2piece[QSTART[q]:QSTART[q] + QSZ[q], :]],
                    outs=[t2all[int(TOFF[q]):int(TOFF[q + 1]), :]],
                )
                add_dep_helper(cc.ins, gnops[q].ins, reason="wave pieces ready")
                ccs.append(cc)
            with (
                tc.tile_pool(name="l1", bufs=4) as l1p,
                tc.tile_pool(name="l1ps", bufs=2, space="PSUM") as l1ps,
                tc.tile_pool(name="l1psh", bufs=2, space="PSUM") as l1psh,
                tc.tile_pool(name="l1psd", bufs=1, space="PSUM") as l1psd,
                tc.tile_pool(name="l1psl", bufs=1, space="PSUM") as l1psl,
                tc.tile_pool(name="l1ps3", bufs=1, space="PSUM") as l1ps3,
            ):
                for ci in range(nchunk):
                    cw = chunk_w[ci]
                    g0 = ci * WCH
                    xq = l1p.tile([128, WCH, 128], BF16, tag="xq")
                    nc.sync.dma_start(xq[:, :cw, :],
                                      xTp_in[:, g0 * 128:(g0 + cw) * 128])
                    ssb = l1p.tile([128, WCH, 128], BF16, tag="ssb")
                    nc.sync.dma_start(ssb[:, :cw, :], S_in[:, g0 * 128:(g0 + cw) * 128])
                    stsb = l1p.tile([128, WCH, 128], BF16, tag="stsb")
                    nc.sync.dma_start(stsb[:, :cw, :], ST_in[:, g0 * 128:(g0 + cw) * 128])
                    for wi in range(cw):
                        g = g0 + wi
                        t = int(win_tile[g])
                        if g in first_win:
                            ps_o = l1ps.tile([128, CH1], F32, tag="ps_o")
                            ps_d = l1psd.tile([128, 8], F32, tag="ps_d")
                        # h_e = x_e @ W1 for this window's 128 edge slots
                        ps_h = l1psh.tile([128, CH1], F32, tag="ps_h")
                        nc.tensor.matmul(ps_h[:], xq[:, wi, :], w1b[:],
                                         start=True, stop=True)
                        # logit = a_s (xq @ Watt_src) + a_d (ST_w.T @ adball),
                        # two matmuls accumulating into the same PSUM region
                        ps_l = l1psl.tile([128, 8], F32, tag="ps_l")
                        nc.tensor.matmul(ps_l[:], xq[:, wi, :],
                                         wattb[:, 0:8], start=True, stop=False)
                        nc.tensor.matmul(ps_l[:], stsb[:TILE_D, wi, :],
                                         adball[:, t, 8:16], start=False, stop=True)
                        ew = l1p.tile([128, 8], F32, tag="ew")
                        nc.scalar.copy(ew[:], ps_l[:])
                        nc.vector.scalar_tensor_tensor(
                            ew[:], ew[:], NEG, ew[:],
                            op0=mybir.AluOpType.mult, op1=mybir.AluOpType.max)
                        expw = l1p.tile([128, 8], BF16, tag="expw")
                        nc.scalar.activation(expw[:], ew[:],
                                             mybir.ActivationFunctionType.Exp)
                        msg = l1p.tile([128, CH1], BF16, tag="msg")
                        eb = expw[:].to_broadcast((128, 8, HID))
                        nc.vector.tensor_mul(
                            msg[:].rearrange("p (h c) -> p h c", h=8),
                            ps_h[:].rearrange("p (h c) -> p h c", h=8), eb)
                        st = g in first_win
                        sp = g in last_win
                        nc.tensor.matmul(ps_o[:], ssb[:, wi, :], msg[:],
                                         start=st, stop=sp)
                        nc.tensor.matmul(ps_d[:], ssb[:, wi, :], expw[:],
                                         start=st, stop=sp)
                        if sp:
                            den = l1p.tile([TILE_D, 8], F32, tag="den")
                            nc.scalar.copy(den[:], ps_d[:TILE_D, :])
                            rec = l1p.tile([TILE_D, 8], F32, tag="rec")
                            nc.vector.reciprocal(rec[:], den[:])
                            x2 = l1p.tile([TILE_D, CH1], F32, tag="x2")
                            rb = rec[:].to_broadcast((TILE_D, 8, HID))
                            nc.vector.tensor_mul(
                                x2[:].rearrange("p (h c) -> p h c", h=8),
                                ps_o[:TILE_D, :].rearrange("p (h c) -> p h c", h=8), rb)
                            nc.vector.tensor_add(x2[:], x2[:], b1bc[:TILE_D, :])
                            x2b = l1p.tile([TILE_D, CH1], BF16, tag="x2b")
                            nc.scalar.activation(x2b[:], x2[:],
                                                 mybir.ActivationFunctionType.Relu)
                            ps_h2 = l1ps3.tile([TILE_D, OUT], F32, tag="ps_h2")
                            for k in range(4):
                                ps_x2t = l1ps3.tile([128, TILE_D], BF16, tag="ps_x2t")
                                nc.tensor.transpose(
                                    ps_x2t[:], x2b[:, k * 128:(k + 1) * 128],
                                    identb[:TILE_D, :TILE_D])
                                x2t = l1p.tile([128, TILE_D], BF16, tag="x2t")
                                nc.scalar.copy(x2t[:], ps_x2t[:])
                                nc.tensor.matmul(ps_h2[:], x2t[:], w2b[:, k, :],
                                                 start=(k == 0), stop=(k == 3))
                            h2 = l1p.tile([TILE_D, OUT], F32, tag="h2")
                            nc.vector.tensor_copy(h2[:], ps_h2[:])
                            tmp = l1p.tile([TILE_D, OUT], F32, tag="tmp")
                            nc.vector.tensor_mul(tmp[:], h2[:], att2sb[:TILE_D, :])
                            nc.vector.tensor_reduce(
                                a2all[:, t, 0:1], tmp[:], op=mybir.AluOpType.add,
                                axis=mybir.AxisListType.X)
                            nc.vector.tensor_mul(tmp[:], h2[:], att2db[:TILE_D, :])
                            nc.vector.tensor_reduce(
                                a2all[:, t, 1:2], tmp[:], op=mybir.AluOpType.add,
                                axis=mybir.AxisListType.X)
                            pc = l1p.tile([TILE_D, 128], BF16, tag="pc")
                            nc.scalar.copy(pc[:, 0:OUT], h2[:])
                            nc.vector.tensor_copy(
                                pc[:, OUT:OUT + 2].bitcast(F32), a2all[:, t, 0:1])
                            tw_ = nc.sync.dma_start(
                                t2piece[t * TILE_D:(t + 1) * TILE_D, :], pc[:])
                            t2_writes.append(tw_.ins)
                            # group-complete marker (fires on the DVE right
                            # after this group's last t2piece write lands);
                            # the AllGather trigger itself is emitted later,
                            # interleaved into the GpSimd gather stream
                            if t in GTILES:
                                gdone = nc.vector.nop()
                                for w_ in t2_writes:
                                    add_dep_helper(gdone.ins, w_,
                                                   reason="group t2 writes")
                                t2_writes = []
                                gnops.append(gdone)

            # ------------- phase 6: layer-2 aggregation -------------
            # pass-major window stream; gathers land in g2all staging and
            # chase the AllGather waves while the PE finishes layer 1.
            # Wave triggers share the GpSimd FIFO with the gathers, so they
            # are interleaved at positions the queue should reach just as
            # each wave's input data lands (estimates; clamped so no chunk
            # ever precedes the wave it waits on).
            # place wave q's trigger RESERVE chunks before the first chunk
            # that waits on it: the reserve keeps the gather queue busy
            # during the wave's flight, and a late-firing wave can only
            # delay its own pass (no compounding)
            RESERVE = 6
            trig_pos = {}
            for q in range(1, NQ):
                lim = min([ci for ci in range(nchunk) if chunk_pass[ci] >= q],
                          default=nchunk)
                prev = trig_pos.get(q - 1, 0)
                trig_pos[q] = max(prev, lim - RESERVE)
            with (
                tc.tile_pool(name="l2", bufs=4) as l2p,
                tc.tile_pool(name="l2ps", bufs=2, space="PSUM") as l2ps,
                tc.tile_pool(name="l2ps2", bufs=2, space="PSUM") as l2ps2,
            ):
                emit_wave(0)
                for ci in range(nchunk):
                    for q in range(1, NQ):
                        if trig_pos.get(q) == ci and len(ccs) == q:
                            emit_wave(q)
                    cw = chunk_w[ci]
                    nidx = cw * 128
                    g0 = ci * WCH
                    ioff = g0 * 8
                    while len(ccs) <= chunk_pass[ci]:
                        emit_wave(len(ccs))
                    gi2 = nc.gpsimd.dma_gather(
                        g2all[:, g0:g0 + cw, :], t2all[:, :],
                        srcidx[:, ioff:ioff + nidx // 16],
                        nidx, nidx, 128)
                    add_dep_helper(gi2.ins, ccs[chunk_pass[ci]].ins,
                                   reason="t2all wave RAW")
                    ssb2 = l2p.tile([128, WCH, 128], BF16, tag="ssb2")
                    nc.sync.dma_start(ssb2[:, :cw, :], S2_in[:, g0 * 128:(g0 + cw) * 128])
                    stsb2 = l2p.tile([128, WCH, 128], BF16, tag="stsb2")
                    nc.sync.dma_start(stsb2[:, :cw, :], ST2_in[:, g0 * 128:(g0 + cw) * 128])
                    for wi in range(cw):
                        j = g0 + wi
                        t = int(tile2[j])
                        if st2[j]:
                            ps_o2 = l2ps.tile([128, OUT + 1], F32, tag="ps_o2")
                            a2b = l2p.tile([TILE_D, 1], BF16, tag="a2b")
                            nc.vector.tensor_copy(a2b[:], a2all[:, t, 1:2])
                        ps_e2 = l2ps2.tile([128, 1], F32, tag="ps_e2")
                        nc.tensor.matmul(ps_e2[:], stsb2[:TILE_D, wi, :], a2b[:],
                                         start=True, stop=True)
                        e2 = l2p.tile([128, 1], F32, tag="e2")
                        nc.vector.tensor_add(e2[:], ps_e2[:],
                                             g2all[:, j, OUT:OUT + 2].bitcast(F32))
                        nc.vector.scalar_tensor_tensor(
                            e2[:], e2[:], NEG, e2[:],
                            op0=mybir.AluOpType.mult, op1=mybir.AluOpType.max)
                        # msg2 = [exp * t2row | exp]: 65 cols so ONE matmul
                        # accumulates numerator and denominator together
                        msg2 = l2p.tile([128, OUT + 1], BF16, tag="msg2")
                        nc.scalar.activation(msg2[:, OUT:OUT + 1], e2[:],
                                             mybir.ActivationFunctionType.Exp)
                        e2b = msg2[:, OUT:OUT + 1].to_broadcast((128, 1, OUT))
                        nc.vector.tensor_mul(
                            msg2[:, 0:OUT].rearrange("p (h c) -> p h c", h=1),
                            g2all[:, j, 0:OUT].rearrange("p (h c) -> p h c", h=1), e2b)
                        nc.tensor.matmul(ps_o2[:], ssb2[:, wi, :], msg2[:],
                                         start=bool(st2[j]), stop=bool(sp2[j]))
                        if sp2[j]:
                            nc.vector.tensor_add(
                                o2acc[:, t, :], o2acc[:, t, :], ps_o2[:TILE_D, :])
                            if fin2[j]:
                                rec2 = l2p.tile([TILE_D, 1], F32, tag="rec2")
                                nc.vector.reciprocal(
                                    rec2[:], o2acc[:, t, OUT:OUT + 1])
                                o2 = l2p.tile([TILE_D, OUT], F32, tag="o2")
                                r2b = rec2[:].to_broadcast((TILE_D, 1, OUT))
                                nc.vector.tensor_mul(
                                    o2[:].rearrange("p (h c) -> p h c", h=1),
                                    o2acc[:, t, 0:OUT].rearrange(
                                        "p (h c) -> p h c", h=1),
                                    r2b)
                                nc.vector.tensor_add(o2[:], o2[:], b2bc[:TILE_D, :])
                                nc.sync.dma_start(
                                    y_out[t * TILE_D:(t + 1) * TILE_D, :], o2[:])

    nc.compile()
    return nc


# --------------------------------------------------------------------- driver
_CACHE = {}


def kernel(x, edge_index, W1, att_src1, att_dst1, b1, W2, att_src2, att_dst2, b2):
    x = np.asarray(x); edge_index = np.asarray(edge_index)
    W1 = np.asarray(W1, np.float32); W2 = np.asarray(W2, np.float32)
    att_src1 = np.asarray(att_src1, np.float32)
    att_dst1 = np.asarray(att_dst1, np.float32)
    att_src2 = np.asarray(att_src2, np.float32)
    att_dst2 = np.asarray(att_dst2, np.float32)
    b1 = np.asarray(b1, np.float32); b2 = np.asarray(b2, np.float32)

    x_bf16 = x.astype(ml_dtypes.bfloat16)
    sched, cores = preprocess(edge_index, x_bf16)
    if "prog" not in _CACHE:
        _CACHE["prog"] = build_program(sched)
    nc = _CACHE["prog"]

    # W_att [128, 16] = per-head reductions of W1 * att1 (host-computed)
    att1 = np.concatenate([att_src1, att_dst1], axis=0)     # [16, 64]
    W1h = W1.reshape(IN, HEADS, HID)
    watt = np.zeros((IN, 16), np.float32)
    for j in range(16):
        watt[:, j] = (W1h[:, j % 8, :] * att1[j][None, :]).sum(axis=1)
    shared = dict(
        W1b=W1.astype(ml_dtypes.bfloat16),
        W2b=np.ascontiguousarray(
            W2.reshape(4, 128, OUT).transpose(1, 0, 2).reshape(128, 4 * OUT)
        ).astype(ml_dtypes.bfloat16),
        wattb=watt.astype(ml_dtypes.bfloat16),
        b1bc=np.broadcast_to(b1.reshape(1, CH1), (128, CH1)).copy(),
        b2bc=np.broadcast_to(b2.reshape(1, OUT), (128, OUT)).copy(),
        att2sb=np.broadcast_to(att_src2.reshape(1, OUT), (128, OUT)).copy(),
        att2db=np.broadcast_to(att_dst2.reshape(1, OUT), (128, OUT)).copy(),
        ident=np.eye(128, dtype=ml_dtypes.bfloat16),
    )
    in_maps = []
    for c in range(NCORES):
        m = dict(shared)
        m["src_idx"] = cores[c]["src_idx"]
        m["S"] = cores[c]["S"]
        m["ST"] = cores[c]["ST"]
        m["S2"] = cores[c]["S2"]
        m["ST2"] = cores[c]["ST2"]
        m["xTp"] = cores[c]["xTp"]
        m["xTown"] = cores[c]["xTown"]
        in_maps.append(m)

    trace = bool(int(os.environ.get("KTRACE", "0")))
    res = run_bass_kernel_spmd(nc, in_maps, core_ids=list(range(NCORES)),
                               trace=trace)
    kernel.last_result = res
    out = np.concatenate([res.results[c]["y"] for c in range(NCORES)], axis=0)
    return out
